# revision 15
# baseline (speedup 1.0000x reference)
"""GQA attention (B=2,S=1024,HID=2048,NH=32,NKV=8,HD=64) on 8 TRN2 cores.

Sharding: core c -> batch b=c//4, head-group g=c%4 (8 q heads / 2 kv heads).
Within a core, q heads are re-paired as (m, m+4) for m in 0..3 so the head
using local kv0 sits at partitions 0:64 and the head using kv1 at 64:128;
K is not replicated.

Dataflow per core (all matmuls bf16 -> fp32 PSUM, N=512):
  front:  Q0 + K projections trail the hsT DMA stream; RoPE via a +-1
          permutation matmul (rotate_half) + cos/sin multiply-add.
  window: the exp stream (ScalarE, ~1.15us per [128,1024] scores tile) runs
          continuously; V proj, Q1-3 projections and their RoPE are emitted
          as PE filler between the scores/PV matmuls so the tensor engine
          works under the exp stream.
  softmax denominators: exp chunks are tree-summed across k-chunks (DVE for
          head A, GpSimd for head B), reduced across partitions with
          gpsimd.partition_all_reduce, inverted with DVE
          reciprocal_approx_fast, and applied while writing attnT.
  tail:   Wo projection, 128 N=512 matmuls with ns-inner ordering so
          consecutive matmuls alternate PSUM banks.

Host pre-lays-out all weights as contiguous [128, N] SBUF images (fast
DMA), folds 1/sqrt(d) into Wq, gathers RoPE tables by position_ids, and
sums the 4 row-parallel Wo partials per batch at the end.

PSUM (8 banks): tag "pv" [128,1024]f32 x2 (proj accumulators / PV pair
accumulators / Wo accumulators) + tag "sc" [128,1024]f32 x2 (score tiles,
rope rotate scratch).
"""

import numpy as np
import ml_dtypes

import concourse.bass as bass
import concourse.bacc as bacc
import concourse.mybir as mybir
import concourse.bass_isa as bass_isa
from concourse.tile import TileContext
from concourse.bass_utils import run_bass_kernel_spmd

B, S, HID = 2, 1024, 2048
NH, NKV, HD = 32, 8, 64
G = 4                      # head groups (tensor-parallel degree per batch)
QH = NH // G               # 8 q heads per core
KVH = NKV // G             # 2 kv heads per core
QD = QH * HD               # 512
HC = HID // 128            # 16 hidden chunks
KC = S // 128              # 8 k-token chunks
ROPE_BASE = 10000.0
BF16 = mybir.dt.bfloat16
F32 = mybir.dt.float32
NEG_BIG = float(np.finfo(np.float32).min)
MULT = mybir.AluOpType.mult
ADD = mybir.AluOpType.add
EXP = mybir.ActivationFunctionType.Exp
COPY = mybir.ActivationFunctionType.Copy

LAST_RESULT = None
_CACHE = {}


def _build(use_mask: bool) -> bass.Bass:
    nc = bacc.Bacc(None, target_bir_lowering=False)
    # weights arrive pre-laid-out as [128, n] SBUF images (contiguous DMA)
    hsT_d = nc.dram_tensor("hsT", [HID, S], BF16, kind="ExternalInput")
    wq_d = nc.dram_tensor("wq", [128, 4 * HC * 128], BF16, kind="ExternalInput")
    wk_d = nc.dram_tensor("wk", [128, HC * 128], BF16, kind="ExternalInput")
    wv_d = nc.dram_tensor("wv", [128, HC * 128], BF16, kind="ExternalInput")
    wo_d = nc.dram_tensor("wo", [128, 4 * HID], BF16, kind="ExternalInput")
    cos_d = nc.dram_tensor("cos2", [128, S], F32, kind="ExternalInput")
    sin_d = nc.dram_tensor("sin2", [128, S], F32, kind="ExternalInput")
    perm_d = nc.dram_tensor("permT", [128, 128], BF16, kind="ExternalInput")
    if use_mask:
        mask_d = nc.dram_tensor("maskT", [S, S], BF16, kind="ExternalInput")
    out_d = nc.dram_tensor("out", [HID, S], F32, kind="ExternalOutput")

    with TileContext(nc) as tc:
        with (
            tc.tile_pool(name="const", bufs=1) as cp,
            tc.tile_pool(name="work", bufs=2) as wp,
            tc.tile_pool(name="ps", bufs=2, space="PSUM") as pp,
        ):
            # warm the exp table + custom-DVE ucode during the DMA window
            dmy = cp.tile([1, 8], F32, tag="dmy")
            nc.any.memset(dmy[:], 1.0)
            dmye = cp.tile([1, 8], BF16, tag="dmye")
            nc.scalar.activation(dmye[:], dmy[:], EXP)
            dmyr = cp.tile([1, 8], F32, tag="dmyr")
            nc.vector.reciprocal_approx_fast(dmyr[:], dmy[:])

            # ---- input DMAs: single queue, priority order ----
            permT = cp.tile([128, 128], BF16, tag="permT")
            nc.sync.dma_start(out=permT[:], in_=perm_d[:, :])
            wkc = cp.tile([128, HC * 128], BF16, tag="wkc")
            nc.sync.dma_start(out=wkc[:], in_=wk_d[:, :])
            wqc = []
            for m in range(4):
                wqc.append(cp.tile([128, HC * 128], BF16, tag=f"wq{m}",
                                   name=f"wq{m}"))
            nc.sync.dma_start(out=wqc[0][:], in_=wq_d[:, 0:HC * 128])
            cos2 = cp.tile([128, S], F32, tag="cos2")
            nc.sync.dma_start(out=cos2[:], in_=cos_d[:, :])
            sin2 = cp.tile([128, S], F32, tag="sin2")
            nc.sync.dma_start(out=sin2[:], in_=sin_d[:, :])
            hsT = []
            for k in range(HC):
                hsT.append(cp.tile([128, S], BF16, tag=f"hsT{k}",
                                   name=f"hsT{k}"))
                nc.sync.dma_start(out=hsT[k][:], in_=hsT_d[k * 128:(k + 1) * 128, :])
            wvc = cp.tile([128, HC * 128], BF16, tag="wvc")
            nc.sync.dma_start(out=wvc[:], in_=wv_d[:, :])
            for m in range(1, 4):
                nc.sync.dma_start(out=wqc[m][:],
                                  in_=wq_d[:, m * HC * 128:(m + 1) * HC * 128])
            woc = cp.tile([128, 4 * HID], BF16, tag="woc")
            nc.sync.dma_start(out=woc[:], in_=wo_d[:, :])
            if use_mask:
                maskT = cp.tile([128, KC * S], BF16, tag="maskT")
                nc.sync.dma_start(
                    out=maskT[:].rearrange("p (k q) -> p k q", k=KC),
                    in_=mask_d[:, :].rearrange("(k p) q -> p k q", p=128),
                )

            # ---- persistent intermediates ----
            krot = cp.tile([128, S], BF16, tag="krot")
            qrot = cp.tile([128, 4 * S], BF16, tag="qrot")
            vtmp = cp.tile([128, S], BF16, tag="vtmp")
            vnat = cp.tile([128, S], BF16, tag="vnat")
            attnT = cp.tile([128, 4 * S], BF16, tag="attnT")

            def rope(ps, dst, use_act):
                """ps: PSUM [128, S] f32 pre-rope; dst: SBUF bf16 [128, S].
                use_act: stage the raw copy on ScalarE (idle pre-exp)."""
                raw = wp.tile([128, S], BF16, tag="raw")
                if use_act:
                    nc.scalar.activation(raw[:], ps[:], COPY)
                else:
                    nc.vector.tensor_copy(raw[:], ps[:])
                rot = pp.tile([128, S], F32, tag="sc")
                for ns in range(2):
                    nc.tensor.matmul(
                        rot[:, ns * 512:(ns + 1) * 512], permT[:],
                        raw[:, ns * 512:(ns + 1) * 512], start=True, stop=True)
                t1 = wp.tile([128, S], F32, tag="t1", bufs=1)
                nc.vector.tensor_tensor(t1[:], raw[:], cos2[:], MULT)
                t2 = wp.tile([128, S], F32, tag="t2", bufs=1)
                nc.vector.tensor_tensor(t2[:], rot[:], sin2[:], MULT)
                nc.vector.tensor_tensor(dst[:], t1[:], t2[:], ADD)

            # ---- Q0 + K projections, interleaved, trailing the hsT DMA ----
            q0ps = pp.tile([128, S], F32, tag="pv")
            kps = pp.tile([128, S], F32, tag="pv")
            for k in range(HC):
                for ns in range(2):
                    nc.tensor.matmul(
                        q0ps[:, ns * 512:(ns + 1) * 512],
                        wqc[0][:, k * 128:(k + 1) * 128],
                        hsT[k][:, ns * 512:(ns + 1) * 512],
                        start=(k == 0), stop=(k == HC - 1),
                    )
                for ns in range(2):
                    nc.tensor.matmul(
                        kps[:, ns * 512:(ns + 1) * 512],
                        wkc[:, k * 128:(k + 1) * 128],
                        hsT[k][:, ns * 512:(ns + 1) * 512],
                        start=(k == 0), stop=(k == HC - 1),
                    )
            rope(q0ps, qrot[:, 0:S], use_act=True)
            rope(kps, krot[:], use_act=True)

            # ---- PE filler units (run under the exp stream) ----
            state = {}

            def v_unit(k):
                def emit():
                    if "vps" not in state:
                        state["vps"] = pp.tile([128, S], F32, tag="pv",
                                               name="vps")
                    vps = state["vps"]
                    for ns in range(2):
                        nc.tensor.matmul(
                            vps[:, ns * 512:(ns + 1) * 512],
                            wvc[:, k * 128:(k + 1) * 128],
                            hsT[k][:, ns * 512:(ns + 1) * 512],
                            start=(k == 0), stop=(k == HC - 1),
                        )
                return emit

            def vnat_unit():
                def emit():
                    nc.vector.tensor_copy(vtmp[:], state["vps"][:])
                    for t in range(KC):
                        nc.sync.dma_start_transpose(
                            vnat[:, t * 128:(t + 1) * 128],
                            vtmp[:, t * 128:(t + 1) * 128],
                        )
                return emit

            def q_unit(m, k):
                def emit():
                    key = f"qps{m}"
                    if key not in state:
                        state[key] = pp.tile([128, S], F32, tag="pv", name=key)
                    qps = state[key]
                    for ns in range(2):
                        nc.tensor.matmul(
                            qps[:, ns * 512:(ns + 1) * 512],
                            wqc[m][:, k * 128:(k + 1) * 128],
                            hsT[k][:, ns * 512:(ns + 1) * 512],
                            start=(k == 0), stop=(k == HC - 1),
                        )
                return emit

            def qrope_unit(m):
                def emit():
                    rope(state[f"qps{m}"], qrot[:, m * S:(m + 1) * S],
                         use_act=False)
                return emit

            filler = {0: [], 1: [], 2: [], 3: []}
            for k in range(HC):
                filler[0].append(v_unit(k))
            filler[0].append(vnat_unit())
            for k in range(HC):
                filler[0].append(q_unit(1, k))
            filler[0].append(qrope_unit(1))
            for m in range(2, 4):
                for k in range(HC):
                    filler[m - 1].append(q_unit(m, k))
                filler[m - 1].append(qrope_unit(m))

            DRAINS = {
                0: [6, 5, 5, 4, 4, 4, 3, 3],   # 34 units: V, vnat, Q1, rope
                1: [4, 3, 3, 3, 2, 2, 0, 0],   # 17 units: Q2, rope
                2: [4, 3, 3, 3, 2, 2, 0, 0],   # 17 units: Q3, rope
                3: [0] * 8,
            }

            def drain(m, n):
                q = filler[m]
                for _ in range(min(n, len(q))):
                    q.pop(0)()

            # ---- attention ----
            exs = {}

            def pv(m, kc, psO):
                exA, exB = exs[(m, kc)]
                for ns in range(2):
                    nc.tensor.matmul(
                        psO[0:64, ns * 512:(ns + 1) * 512],
                        vnat[:, kc * 128:kc * 128 + 64],
                        exA[:, ns * 512:(ns + 1) * 512],
                        start=(kc == 0), stop=(kc == KC - 1),
                    )
                for ns in range(2):
                    nc.tensor.matmul(
                        psO[64:128, ns * 512:(ns + 1) * 512],
                        vnat[:, kc * 128 + 64:(kc + 1) * 128],
                        exB[:, ns * 512:(ns + 1) * 512],
                        start=(kc == 0), stop=(kc == KC - 1),
                    )

            def finish_pair(m, psO, u1):
                """PV tail + denominators + normalize for pair m -> attnT."""
                for j in range(6, KC):
                    pv(m, j, psO)
                for h in range(2):
                    red = wp.tile([128, S], F32, tag="red", bufs=1)
                    nc.gpsimd.partition_all_reduce(
                        red[:], u1[h][:], channels=128,
                        reduce_op=bass_isa.ReduceOp.add)
                    rcp = wp.tile([128, S], F32, tag="rcp")
                    nc.vector.reciprocal_approx_fast(rcp[:], red[:])
                    r = h * 64
                    nc.vector.tensor_tensor(
                        attnT[r:r + 64, m * S:(m + 1) * S],
                        psO[r:r + 64, :], rcp[r:r + 64, :], MULT)

            pending = None
            for m in range(4):
                psO = None
                u1 = {}
                u3 = {}
                for kc in range(KC):
                    scA = pp.tile([128, S], F32, tag="sc")
                    scB = pp.tile([128, S], F32, tag="sc")
                    for ns in range(2):
                        nc.tensor.matmul(
                            scA[:, ns * 512:(ns + 1) * 512],
                            krot[0:64, kc * 128:(kc + 1) * 128],
                            qrot[0:64, m * S + ns * 512:m * S + ns * 512 + 512],
                            start=True, stop=True,
                        )
                    for ns in range(2):
                        nc.tensor.matmul(
                            scB[:, ns * 512:(ns + 1) * 512],
                            krot[64:128, kc * 128:(kc + 1) * 128],
                            qrot[64:128, m * S + ns * 512:m * S + ns * 512 + 512],
                            start=True, stop=True,
                        )
                    if use_mask:
                        nc.vector.tensor_tensor(
                            scA[:], scA[:], maskT[:, kc * S:(kc + 1) * S], ADD)
                        nc.vector.tensor_tensor(
                            scB[:], scB[:], maskT[:, kc * S:(kc + 1) * S], ADD)
                    exA = wp.tile([128, S], BF16, tag="ex", bufs=16)
                    nc.scalar.activation(exA[:], scA[:], EXP)
                    exB = wp.tile([128, S], BF16, tag="ex", bufs=16)
                    nc.scalar.activation(exB[:], scB[:], EXP)
                    exs[(m, kc)] = (exA, exB)

                    # previous pair's tail runs after this pair's exp stream
                    # is already fed, keeping ScalarE busy across the seam
                    if kc == 0 and pending is not None:
                        finish_pair(*pending)
                        pending = None

                    # incremental tree-sum of exp chunks (denominators);
                    # head A on DVE, head B on GpSimd
                    eng = (nc.vector, nc.gpsimd)
                    if kc in (1, 5):
                        tgt = u1 if kc == 1 else u3
                        for h in range(2):
                            t = wp.tile([128, S], BF16, tag="tt", bufs=8)
                            eng[h].tensor_tensor(
                                t[:], exs[(m, kc - 1)][h][:], exs[(m, kc)][h][:],
                                ADD)
                            tgt[h] = t
                    if kc in (3, 7):
                        tgt = u1 if kc == 3 else u3
                        for h in range(2):
                            t = wp.tile([128, S], BF16, tag="tt", bufs=8)
                            eng[h].tensor_tensor(
                                t[:], exs[(m, kc - 1)][h][:], exs[(m, kc)][h][:],
                                ADD)
                            eng[h].tensor_tensor(
                                tgt[h][:], tgt[h][:], t[:], ADD)
                    if kc == 7:
                        for h in range(2):
                            eng[h].tensor_tensor(
                                u1[h][:], u1[h][:], u3[h][:], ADD)

                    # PV lags the exp stream; pair 0 also waits for V/vnat
                    if m == 0:
                        if kc == 6:
                            psO = pp.tile([128, S], F32, tag="pv")
                            for j in range(3):
                                pv(m, j, psO)
                        elif kc == 7:
                            for j in range(3, 6):
                                pv(m, j, psO)
                    elif kc >= 2:
                        if psO is None:
                            psO = pp.tile([128, S], F32, tag="pv")
                        pv(m, kc - 2, psO)

                    drain(m, DRAINS[m][kc])

                drain(m, len(filler[m]))
                pending = (m, psO, u1)

            finish_pair(*pending)

            # ---- output projection (ns-inner so consecutive matmuls
            # alternate PSUM banks and pipeline) ----
            for mc2 in range(HC):
                psW = pp.tile([128, S], F32, tag="pv")
                for mm in range(4):
                    for ns in range(2):
                        nc.tensor.matmul(
                            psW[:, ns * 512:(ns + 1) * 512],
                            woc[:, mm * HID + mc2 * 128:mm * HID + (mc2 + 1) * 128],
                            attnT[:, mm * S + ns * 512:mm * S + ns * 512 + 512],
                            start=(mm == 0), stop=(mm == 3),
                        )
                outst = wp.tile([128, S], F32, tag="os")
                nc.vector.tensor_copy(outst[:], psW[:])
                nc.sync.dma_start(
                    out=out_d[mc2 * 128:(mc2 + 1) * 128, :], in_=outst[:])
    nc.finalize()
    return nc


def _rope_tables():
    inv = 1.0 / (ROPE_BASE ** (np.arange(0, HD, 2, dtype=np.float32) / HD))
    t = np.arange(S, dtype=np.float32)
    freqs = np.outer(t, inv)
    emb = np.concatenate([freqs, freqs], axis=-1)  # [S, HD]
    return np.cos(emb).astype(np.float32), np.sin(emb).astype(np.float32)


def _perm_T():
    P = np.zeros((128, 128), dtype=np.float32)
    for blk in range(2):
        o = blk * 64
        for i in range(32):
            P[o + i, o + i + 32] = -1.0
            P[o + i + 32, o + i] = 1.0
    return P.T.astype(ml_dtypes.bfloat16)


def _img(w):
    """[HC*128, n] weight -> [128, HC*n] SBUF image (k-chunk-major)."""
    k = w.shape[0] // 128
    return np.ascontiguousarray(
        w.reshape(k, 128, w.shape[1]).transpose(1, 0, 2).reshape(128, -1))


def kernel(hidden_states, position_ids, attention_mask, Wq, Wk, Wv, Wo,
           _trace=False):
    global LAST_RESULT
    bf = ml_dtypes.bfloat16
    hidden_states = np.asarray(hidden_states, dtype=np.float32)
    Wq = np.asarray(Wq, dtype=np.float32)
    Wk = np.asarray(Wk, dtype=np.float32)
    Wv = np.asarray(Wv, dtype=np.float32)
    Wo = np.asarray(Wo, dtype=np.float32)
    mask = np.asarray(attention_mask, dtype=np.float32)
    pos = np.asarray(position_ids).astype(np.int64)

    use_mask = bool(np.any(mask))
    if use_mask not in _CACHE:
        _CACHE[use_mask] = _build(use_mask)
    nc = _CACHE[use_mask]

    cos_t, sin_t = _rope_tables()
    permT = _perm_T()
    scale = 1.0 / np.sqrt(HD)

    in_maps = []
    for c in range(8):
        b, g = c // G, c % G
        # paired head order: chunk m holds (head 8g+m, head 8g+m+4)
        order = []
        for m in range(4):
            order += [8 * g + m, 8 * g + m + 4]
        wq_g = np.concatenate(
            [Wq[:, h * HD:(h + 1) * HD] for h in order], axis=1) * scale
        # wq image: 4 chunk-images side by side
        wq_im = np.concatenate(
            [_img(wq_g[:, m * 128:(m + 1) * 128]) for m in range(4)], axis=1)
        wo_g = np.concatenate(
            [Wo[h * HD:(h + 1) * HD, :] for h in order], axis=0)
        kv0 = 2 * g * HD
        m = {
            "hsT": np.ascontiguousarray(hidden_states[b].T).astype(bf),
            "wq": wq_im.astype(bf),
            "wk": _img(Wk[:, kv0:kv0 + 2 * HD]).astype(bf),
            "wv": _img(Wv[:, kv0:kv0 + 2 * HD]).astype(bf),
            "wo": _img(wo_g).astype(bf),
            "permT": permT,
            "cos2": np.ascontiguousarray(
                np.tile(cos_t[pos[b]].T, (2, 1))).astype(np.float32),
            "sin2": np.ascontiguousarray(
                np.tile(sin_t[pos[b]].T, (2, 1))).astype(np.float32),
        }
        if use_mask:
            mt = np.maximum(mask[b, 0], NEG_BIG).T    # [S(k), S(q)]
            m["maskT"] = np.ascontiguousarray(mt).astype(bf)
        in_maps.append(m)

    res = run_bass_kernel_spmd(nc, in_maps, core_ids=list(range(8)),
                               trace=_trace)
    LAST_RESULT = res
    out = np.zeros((B, S, HID), dtype=np.float32)
    for c in range(8):
        out[c // G] += res.results[c]["out"].T
    return out


# revision 18
# speedup vs baseline: 1.9305x; 1.9305x over previous
"""GQA attention (B=2,S=1024,HID=2048,NH=32,NKV=8,HD=64) on 8 TRN2 cores.

Sharding: core c -> batch b=c//4, head-group g=c%4 (8 q heads / 2 kv heads).
Within a core, q heads are re-paired as (m, m+4) for m in 0..3 so the head
using local kv0 sits at partitions 0:64 and the head using kv1 at 64:128;
K is not replicated.

Dataflow per core (all matmuls bf16 -> fp32 PSUM, N=512):
  front:  Q0 + K projections trail the hsT DMA stream; RoPE via a +-1
          permutation matmul (rotate_half) + cos/sin multiply-add.
  window: the exp stream (ScalarE, ~1.15us per [128,1024] scores tile) runs
          continuously; V proj, Q1-3 projections and their RoPE are emitted
          as PE filler between the scores/PV matmuls so the tensor engine
          works under the exp stream.
  softmax denominators: exp chunks are tree-summed across k-chunks (DVE for
          head A, GpSimd for head B), reduced across partitions with
          gpsimd.partition_all_reduce, inverted with DVE
          reciprocal_approx_fast, and applied while writing attnT.
  tail:   Wo projection, 128 N=512 matmuls with ns-inner ordering so
          consecutive matmuls alternate PSUM banks.

Host pre-lays-out all weights as contiguous [128, N] SBUF images (fast
DMA), folds 1/sqrt(d) into Wq, gathers RoPE tables by position_ids, and
sums the 4 row-parallel Wo partials per batch at the end.

PSUM (8 banks): tag "pv" [128,1024]f32 x2 (proj accumulators / PV pair
accumulators / Wo accumulators) + tag "sc" [128,1024]f32 x2 (score tiles,
rope rotate scratch).
"""

import numpy as np
import ml_dtypes

import concourse.bass as bass
import concourse.bacc as bacc
import concourse.mybir as mybir
import concourse.bass_isa as bass_isa
from concourse.tile import TileContext
from concourse.bass_utils import run_bass_kernel_spmd

B, S, HID = 2, 1024, 2048
NH, NKV, HD = 32, 8, 64
G = 4                      # head groups (tensor-parallel degree per batch)
QH = NH // G               # 8 q heads per core
KVH = NKV // G             # 2 kv heads per core
QD = QH * HD               # 512
HC = HID // 128            # 16 hidden chunks
KC = S // 128              # 8 k-token chunks
ROPE_BASE = 10000.0
BF16 = mybir.dt.bfloat16
F32 = mybir.dt.float32
NEG_BIG = float(np.finfo(np.float32).min)
MULT = mybir.AluOpType.mult
ADD = mybir.AluOpType.add
EXP = mybir.ActivationFunctionType.Exp
COPY = mybir.ActivationFunctionType.Copy

LAST_RESULT = None
_CACHE = {}


def _build(use_mask: bool) -> bass.Bass:
    nc = bacc.Bacc(None, target_bir_lowering=False)
    # weights arrive pre-laid-out as [128, n] SBUF images (contiguous DMA)
    hsT_d = nc.dram_tensor("hsT", [HID, S], BF16, kind="ExternalInput")
    wq_d = nc.dram_tensor("wq", [128, 4 * HC * 128], BF16, kind="ExternalInput")
    wk_d = nc.dram_tensor("wk", [128, HC * 128], BF16, kind="ExternalInput")
    wv_d = nc.dram_tensor("wv", [128, HC * 128], BF16, kind="ExternalInput")
    wo_d = nc.dram_tensor("wo", [128, 4 * HID], BF16, kind="ExternalInput")
    cos_d = nc.dram_tensor("cos2", [128, S], F32, kind="ExternalInput")
    sin_d = nc.dram_tensor("sin2", [128, S], F32, kind="ExternalInput")
    perm_d = nc.dram_tensor("permT", [128, 128], BF16, kind="ExternalInput")
    if use_mask:
        mask_d = nc.dram_tensor("maskT", [S, S], BF16, kind="ExternalInput")
    out_d = nc.dram_tensor("out", [HID, S], F32, kind="ExternalOutput")

    with TileContext(nc) as tc:
        with (
            tc.tile_pool(name="const", bufs=1) as cp,
            tc.tile_pool(name="work", bufs=2) as wp,
            tc.tile_pool(name="ps", bufs=2, space="PSUM") as pp,
        ):
            # warm the exp table + custom-DVE ucode during the DMA window
            dmy = cp.tile([1, 8], F32, tag="dmy")
            nc.any.memset(dmy[:], 1.0)
            dmye = cp.tile([1, 8], BF16, tag="dmye")
            nc.scalar.activation(dmye[:], dmy[:], EXP)
            dmyr = cp.tile([1, 8], F32, tag="dmyr")
            nc.vector.reciprocal_approx_fast(dmyr[:], dmy[:])

            ones_col = cp.tile([128, 1], BF16, tag="ones")
            nc.any.memset(ones_col[:], 1.0)

            # ---- input DMAs: single queue, priority order ----
            permT = cp.tile([128, 128], BF16, tag="permT")
            nc.sync.dma_start(out=permT[:], in_=perm_d[:, :])
            wkc = cp.tile([128, HC * 128], BF16, tag="wkc")
            nc.sync.dma_start(out=wkc[:], in_=wk_d[:, :])
            wqc = []
            for m in range(4):
                wqc.append(cp.tile([128, HC * 128], BF16, tag=f"wq{m}",
                                   name=f"wq{m}"))
            nc.sync.dma_start(out=wqc[0][:], in_=wq_d[:, 0:HC * 128])
            cos2 = cp.tile([128, S], F32, tag="cos2")
            nc.sync.dma_start(out=cos2[:], in_=cos_d[:, :])
            sin2 = cp.tile([128, S], F32, tag="sin2")
            nc.sync.dma_start(out=sin2[:], in_=sin_d[:, :])
            hsT = []
            for k in range(HC):
                hsT.append(cp.tile([128, S], BF16, tag=f"hsT{k}",
                                   name=f"hsT{k}"))
                nc.sync.dma_start(out=hsT[k][:], in_=hsT_d[k * 128:(k + 1) * 128, :])
            wvc = cp.tile([128, HC * 128], BF16, tag="wvc")
            nc.sync.dma_start(out=wvc[:], in_=wv_d[:, :])
            for m in range(1, 4):
                nc.sync.dma_start(out=wqc[m][:],
                                  in_=wq_d[:, m * HC * 128:(m + 1) * HC * 128])
            woc = cp.tile([128, 4 * HID], BF16, tag="woc")
            nc.sync.dma_start(out=woc[:], in_=wo_d[:, :])
            if use_mask:
                maskT = cp.tile([128, KC * S], BF16, tag="maskT")
                nc.sync.dma_start(
                    out=maskT[:].rearrange("p (k q) -> p k q", k=KC),
                    in_=mask_d[:, :].rearrange("(k p) q -> p k q", p=128),
                )

            # ---- persistent intermediates ----
            krot = cp.tile([128, S], BF16, tag="krot")
            qrot = cp.tile([128, 4 * S], BF16, tag="qrot")
            vtmp = cp.tile([128, S], BF16, tag="vtmp")
            vnat = cp.tile([128, S], BF16, tag="vnat")
            attnT = cp.tile([128, 4 * S], BF16, tag="attnT")

            def rope(ps, dst, use_act):
                """ps: PSUM [128, S] f32 pre-rope; dst: SBUF bf16 [128, S].
                use_act: stage the raw copy on ScalarE (idle pre-exp)."""
                raw = wp.tile([128, S], BF16, tag="raw")
                if use_act:
                    nc.scalar.activation(raw[:], ps[:], COPY)
                else:
                    nc.vector.tensor_copy(raw[:], ps[:])
                rot = pp.tile([128, S], F32, tag="sc")
                for ns in range(2):
                    nc.tensor.matmul(
                        rot[:, ns * 512:(ns + 1) * 512], permT[:],
                        raw[:, ns * 512:(ns + 1) * 512], start=True, stop=True)
                t1 = wp.tile([128, S], F32, tag="t1", bufs=1)
                nc.vector.tensor_tensor(t1[:], raw[:], cos2[:], MULT)
                t2 = wp.tile([128, S], F32, tag="t2", bufs=1)
                nc.vector.tensor_tensor(t2[:], rot[:], sin2[:], MULT)
                nc.vector.tensor_tensor(dst[:], t1[:], t2[:], ADD)

            # ---- Q0 + K projections, interleaved, trailing the hsT DMA ----
            q0ps = pp.tile([128, S], F32, tag="pv")
            kps = pp.tile([128, S], F32, tag="pv")
            for k in range(HC):
                for ns in range(2):
                    nc.tensor.matmul(
                        q0ps[:, ns * 512:(ns + 1) * 512],
                        wqc[0][:, k * 128:(k + 1) * 128],
                        hsT[k][:, ns * 512:(ns + 1) * 512],
                        start=(k == 0), stop=(k == HC - 1),
                    )
                for ns in range(2):
                    nc.tensor.matmul(
                        kps[:, ns * 512:(ns + 1) * 512],
                        wkc[:, k * 128:(k + 1) * 128],
                        hsT[k][:, ns * 512:(ns + 1) * 512],
                        start=(k == 0), stop=(k == HC - 1),
                    )
            rope(q0ps, qrot[:, 0:S], use_act=True)
            rope(kps, krot[:], use_act=True)

            # ---- PE filler units (run under the exp stream) ----
            state = {}

            def v_unit(k):
                def emit():
                    if "vps" not in state:
                        state["vps"] = pp.tile([128, S], F32, tag="pv",
                                               name="vps")
                    vps = state["vps"]
                    for ns in range(2):
                        nc.tensor.matmul(
                            vps[:, ns * 512:(ns + 1) * 512],
                            wvc[:, k * 128:(k + 1) * 128],
                            hsT[k][:, ns * 512:(ns + 1) * 512],
                            start=(k == 0), stop=(k == HC - 1),
                        )
                return emit

            def vnat_unit():
                def emit():
                    nc.vector.tensor_copy(vtmp[:], state["vps"][:])
                    for t in range(KC):
                        nc.sync.dma_start_transpose(
                            vnat[:, t * 128:(t + 1) * 128],
                            vtmp[:, t * 128:(t + 1) * 128],
                        )
                return emit

            def q_unit(m, k):
                def emit():
                    key = f"qps{m}"
                    if key not in state:
                        state[key] = pp.tile([128, S], F32, tag="pv", name=key)
                    qps = state[key]
                    for ns in range(2):
                        nc.tensor.matmul(
                            qps[:, ns * 512:(ns + 1) * 512],
                            wqc[m][:, k * 128:(k + 1) * 128],
                            hsT[k][:, ns * 512:(ns + 1) * 512],
                            start=(k == 0), stop=(k == HC - 1),
                        )
                return emit

            def qrope_unit(m):
                def emit():
                    rope(state[f"qps{m}"], qrot[:, m * S:(m + 1) * S],
                         use_act=False)
                return emit

            filler = {0: [], 1: [], 2: [], 3: []}
            for k in range(HC):
                filler[0].append(v_unit(k))
            filler[0].append(vnat_unit())
            for k in range(HC):
                filler[0].append(q_unit(1, k))
            filler[0].append(qrope_unit(1))
            for m in range(2, 4):
                for k in range(HC):
                    filler[m - 1].append(q_unit(m, k))
                filler[m - 1].append(qrope_unit(m))

            DRAINS = {
                0: [6, 5, 5, 4, 4, 4, 3, 3],   # 34 units: V, vnat, Q1, rope
                1: [4, 3, 3, 3, 2, 2, 0, 0],   # 17 units: Q2, rope
                2: [4, 3, 3, 3, 2, 2, 0, 0],   # 17 units: Q3, rope
                3: [0] * 8,
            }

            def drain(m, n):
                q = filler[m]
                for _ in range(min(n, len(q))):
                    q.pop(0)()

            # ---- attention ----
            exs = {}

            def pv(m, kc, psO):
                exA, exB = exs[(m, kc)]
                for ns in range(2):
                    nc.tensor.matmul(
                        psO[0:64, ns * 512:(ns + 1) * 512],
                        vnat[:, kc * 128:kc * 128 + 64],
                        exA[:, ns * 512:(ns + 1) * 512],
                        start=(kc == 0), stop=(kc == KC - 1),
                    )
                for ns in range(2):
                    nc.tensor.matmul(
                        psO[64:128, ns * 512:(ns + 1) * 512],
                        vnat[:, kc * 128 + 64:(kc + 1) * 128],
                        exB[:, ns * 512:(ns + 1) * 512],
                        start=(kc == 0), stop=(kc == KC - 1),
                    )

            def finish_pair(m, psO, u1):
                """PV tail + denominators + normalize for pair m -> attnT."""
                for j in range(6, KC):
                    pv(m, j, psO)
                rcs = []
                for h in range(2):
                    for half in range(2):
                        dn = pp.tile([1, 512], F32, tag="sc")
                        nc.tensor.matmul(
                            dn[:], ones_col[:],
                            u1[h][:, half * 512:(half + 1) * 512],
                            start=True, stop=True,
                        )
                        rc = wp.tile([1, 512], F32, tag="rc", bufs=4)
                        nc.vector.reciprocal_approx_fast(rc[:], dn[:])
                        rcs.append(rc)
                for h in range(2):
                    bc = wp.tile([128, S], F32, tag="bc")
                    nc.gpsimd.partition_broadcast(bc[:, 0:512], rcs[2 * h][:])
                    nc.gpsimd.partition_broadcast(
                        bc[:, 512:1024], rcs[2 * h + 1][:])
                    r = h * 64
                    nc.vector.tensor_tensor(
                        attnT[r:r + 64, m * S:(m + 1) * S],
                        psO[r:r + 64, :], bc[r:r + 64, :], MULT)

            pending = None
            for m in range(4):
                psO = None
                u1 = {}
                u3 = {}
                for kc in range(KC):
                    scA = pp.tile([128, S], F32, tag="sc")
                    scB = pp.tile([128, S], F32, tag="sc")
                    for ns in range(2):
                        nc.tensor.matmul(
                            scA[:, ns * 512:(ns + 1) * 512],
                            krot[0:64, kc * 128:(kc + 1) * 128],
                            qrot[0:64, m * S + ns * 512:m * S + ns * 512 + 512],
                            start=True, stop=True,
                        )
                    for ns in range(2):
                        nc.tensor.matmul(
                            scB[:, ns * 512:(ns + 1) * 512],
                            krot[64:128, kc * 128:(kc + 1) * 128],
                            qrot[64:128, m * S + ns * 512:m * S + ns * 512 + 512],
                            start=True, stop=True,
                        )
                    if use_mask:
                        nc.vector.tensor_tensor(
                            scA[:], scA[:], maskT[:, kc * S:(kc + 1) * S], ADD)
                        nc.vector.tensor_tensor(
                            scB[:], scB[:], maskT[:, kc * S:(kc + 1) * S], ADD)
                    exA = wp.tile([128, S], BF16, tag="ex", bufs=16)
                    nc.scalar.activation(exA[:], scA[:], EXP)
                    exB = wp.tile([128, S], BF16, tag="ex", bufs=16)
                    nc.scalar.activation(exB[:], scB[:], EXP)
                    exs[(m, kc)] = (exA, exB)

                    # previous pair's tail runs after this pair's exp stream
                    # is already fed, keeping ScalarE busy across the seam
                    if kc == 0 and pending is not None:
                        finish_pair(*pending)
                        pending = None

                    # incremental tree-sum of exp chunks (denominators)
                    eng = (nc.vector, nc.vector)
                    if kc in (1, 5):
                        tgt = u1 if kc == 1 else u3
                        for h in range(2):
                            t = wp.tile([128, S], BF16, tag="tt", bufs=8)
                            eng[h].tensor_tensor(
                                t[:], exs[(m, kc - 1)][h][:], exs[(m, kc)][h][:],
                                ADD)
                            tgt[h] = t
                    if kc in (3, 7):
                        tgt = u1 if kc == 3 else u3
                        for h in range(2):
                            t = wp.tile([128, S], BF16, tag="tt", bufs=8)
                            eng[h].tensor_tensor(
                                t[:], exs[(m, kc - 1)][h][:], exs[(m, kc)][h][:],
                                ADD)
                            eng[h].tensor_tensor(
                                tgt[h][:], tgt[h][:], t[:], ADD)
                    if kc == 7:
                        for h in range(2):
                            eng[h].tensor_tensor(
                                u1[h][:], u1[h][:], u3[h][:], ADD)

                    # PV lags the exp stream; pair 0 also waits for V/vnat
                    if m == 0:
                        if kc == 6:
                            psO = pp.tile([128, S], F32, tag="pv")
                            for j in range(3):
                                pv(m, j, psO)
                        elif kc == 7:
                            for j in range(3, 6):
                                pv(m, j, psO)
                    elif kc >= 2:
                        if psO is None:
                            psO = pp.tile([128, S], F32, tag="pv")
                        pv(m, kc - 2, psO)

                    drain(m, DRAINS[m][kc])

                drain(m, len(filler[m]))
                pending = (m, psO, u1)

            finish_pair(*pending)

            # ---- output projection (ns-inner so consecutive matmuls
            # alternate PSUM banks and pipeline) ----
            for mc2 in range(HC):
                psW = pp.tile([128, S], F32, tag="pv")
                for mm in range(4):
                    for ns in range(2):
                        nc.tensor.matmul(
                            psW[:, ns * 512:(ns + 1) * 512],
                            woc[:, mm * HID + mc2 * 128:mm * HID + (mc2 + 1) * 128],
                            attnT[:, mm * S + ns * 512:mm * S + ns * 512 + 512],
                            start=(mm == 0), stop=(mm == 3),
                        )
                outst = wp.tile([128, S], F32, tag="os")
                nc.vector.tensor_copy(outst[:], psW[:])
                nc.sync.dma_start(
                    out=out_d[mc2 * 128:(mc2 + 1) * 128, :], in_=outst[:])
    nc.finalize()
    return nc


def _rope_tables():
    inv = 1.0 / (ROPE_BASE ** (np.arange(0, HD, 2, dtype=np.float32) / HD))
    t = np.arange(S, dtype=np.float32)
    freqs = np.outer(t, inv)
    emb = np.concatenate([freqs, freqs], axis=-1)  # [S, HD]
    return np.cos(emb).astype(np.float32), np.sin(emb).astype(np.float32)


def _perm_T():
    P = np.zeros((128, 128), dtype=np.float32)
    for blk in range(2):
        o = blk * 64
        for i in range(32):
            P[o + i, o + i + 32] = -1.0
            P[o + i + 32, o + i] = 1.0
    return P.T.astype(ml_dtypes.bfloat16)


def _img(w):
    """[HC*128, n] weight -> [128, HC*n] SBUF image (k-chunk-major)."""
    k = w.shape[0] // 128
    return np.ascontiguousarray(
        w.reshape(k, 128, w.shape[1]).transpose(1, 0, 2).reshape(128, -1))


def kernel(hidden_states, position_ids, attention_mask, Wq, Wk, Wv, Wo,
           _trace=False):
    global LAST_RESULT
    bf = ml_dtypes.bfloat16
    hidden_states = np.asarray(hidden_states, dtype=np.float32)
    Wq = np.asarray(Wq, dtype=np.float32)
    Wk = np.asarray(Wk, dtype=np.float32)
    Wv = np.asarray(Wv, dtype=np.float32)
    Wo = np.asarray(Wo, dtype=np.float32)
    mask = np.asarray(attention_mask, dtype=np.float32)
    pos = np.asarray(position_ids).astype(np.int64)

    use_mask = bool(np.any(mask))
    if use_mask not in _CACHE:
        _CACHE[use_mask] = _build(use_mask)
    nc = _CACHE[use_mask]

    cos_t, sin_t = _rope_tables()
    permT = _perm_T()
    scale = 1.0 / np.sqrt(HD)

    in_maps = []
    for c in range(8):
        b, g = c // G, c % G
        # paired head order: chunk m holds (head 8g+m, head 8g+m+4)
        order = []
        for m in range(4):
            order += [8 * g + m, 8 * g + m + 4]
        wq_g = np.concatenate(
            [Wq[:, h * HD:(h + 1) * HD] for h in order], axis=1) * scale
        # wq image: 4 chunk-images side by side
        wq_im = np.concatenate(
            [_img(wq_g[:, m * 128:(m + 1) * 128]) for m in range(4)], axis=1)
        wo_g = np.concatenate(
            [Wo[h * HD:(h + 1) * HD, :] for h in order], axis=0)
        kv0 = 2 * g * HD
        m = {
            "hsT": np.ascontiguousarray(hidden_states[b].T).astype(bf),
            "wq": wq_im.astype(bf),
            "wk": _img(Wk[:, kv0:kv0 + 2 * HD]).astype(bf),
            "wv": _img(Wv[:, kv0:kv0 + 2 * HD]).astype(bf),
            "wo": _img(wo_g).astype(bf),
            "permT": permT,
            "cos2": np.ascontiguousarray(
                np.tile(cos_t[pos[b]].T, (2, 1))).astype(np.float32),
            "sin2": np.ascontiguousarray(
                np.tile(sin_t[pos[b]].T, (2, 1))).astype(np.float32),
        }
        if use_mask:
            mt = np.maximum(mask[b, 0], NEG_BIG).T    # [S(k), S(q)]
            m["maskT"] = np.ascontiguousarray(mt).astype(bf)
        in_maps.append(m)

    res = run_bass_kernel_spmd(nc, in_maps, core_ids=list(range(8)),
                               trace=_trace)
    LAST_RESULT = res
    out = np.zeros((B, S, HID), dtype=np.float32)
    for c in range(8):
        out[c // G] += res.results[c]["out"].T
    return out
